# revision 3
# baseline (speedup 1.0000x reference)
"""MistralAudioCodebook TRN2 kernel (nn_MistralAudioCodebook_88656714924740).

Data-parallel over batch: 8 batches -> 8 NeuronCores (1 batch each).

Per core (x_b [292, 4096]):
  sem:  scores[t, k] = 2*x_t.e_k - |e_k|^2 computed on PE as a 3-pass
        split-precision matmul (fp32r hi/lo on x, fp32r hi + bf16 lo on E,
        quad-bf16 e2 added last) accumulating in PSUM fp32; argmax via
        DVE prefix-max scan + ACT Sign(gmax-c) accumulate (= index of first
        max, matching jnp.argmin tie semantics); decode via indirect-DMA
        gather of emb rows + PE transpose.
  ac:   FSQ tanh -> affine -> round-half-even (magic-number add) -> codes
        and affine decode.

Self-contained: hardcodes all shapes from the problem spec.
"""
import numpy as np

import concourse.bass as bass
import concourse.bacc as bacc
import concourse.tile as tile
import concourse.mybir as mybir
from concourse import masks
from concourse.bass import IndirectOffsetOnAxis
from concourse.bass_utils import run_bass_kernel_spmd

F32 = mybir.dt.float32
F32R = mybir.dt.float32r
BF16 = mybir.dt.bfloat16
I32 = mybir.dt.int32
AF = mybir.ActivationFunctionType
ALU = mybir.AluOpType

B = 8
D = 292
T = 4096
S = 256          # SEM_DIM
K = 8192         # CODEBOOK_SIZE
A = D - S        # 36 acoustic dims
FSQ = 21
EPS = 1e-5

NTILES = T // 128          # 32 sample tiles
NQ = 8                     # quarters per tile (1024 codes each)
QW = K // NQ               # 1024
AC_FLAT_P = 128
AC_FLAT_F = (A * T) // 128  # 1152
AC_THIRD = AC_FLAT_F // 3   # 384
MAGIC = 8388608.0          # 2^23: (x + 2^23) - 2^23 == round-half-even(x)


def _build(nc: bass.Bass):
    xb = nc.dram_tensor("xb", [D, T], F32, kind="ExternalInput")
    e2t = nc.dram_tensor("e2t", [S, K], F32, kind="ExternalInput")    # 2*emb^T
    e2q = nc.dram_tensor("e2q", [4, K], BF16, kind="ExternalInput")   # -|e|^2 quad bf16 split
    embt = nc.dram_tensor("embt", [K, S], F32, kind="ExternalInput")  # emb rows (gather)

    o_csem = nc.dram_tensor("o_csem", [T], I32, kind="ExternalOutput")
    o_cac = nc.dram_tensor("o_cac", [AC_FLAT_P, AC_FLAT_F], I32, kind="ExternalOutput")
    o_rsem = nc.dram_tensor("o_rsem", [S, T], F32, kind="ExternalOutput")
    o_rac = nc.dram_tensor("o_rac", [AC_FLAT_P, AC_FLAT_F], F32, kind="ExternalOutput")

    with tile.TileContext(nc) as tc:
        with (
            tc.tile_pool(name="const", bufs=1) as cp,
            tc.tile_pool(name="stage", bufs=2) as stp,
            tc.tile_pool(name="xp", bufs=2) as xp,
            tc.tile_pool(name="cp_sc", bufs=1) as ccp,
            tc.tile_pool(name="sgn", bufs=1) as sgp,
            tc.tile_pool(name="sm", bufs=4) as smp,
            tc.tile_pool(name="gp", bufs=2) as gp,
            tc.tile_pool(name="acp", bufs=1) as acp,
            tc.tile_pool(name="ps", bufs=3, space="PSUM") as ps,
            tc.tile_pool(name="pst", bufs=2, space="PSUM") as pst,
        ):
            # ---------------- one-time constants ----------------
            er = cp.tile([128, 2, K], F32R, tag="er")
            el = cp.tile([128, 2, K], BF16, tag="el")
            for h in range(2):
                for cc in range(K // 512):
                    sl = slice(cc * 512, (cc + 1) * 512)
                    ef = stp.tile([128, 512], F32, tag="estage")
                    nc.sync.dma_start(ef[:], e2t[h * 128 : (h + 1) * 128, sl])
                    nc.scalar.copy(er[:, h, sl], ef[:])
                    nc.vector.tensor_tensor(
                        el[:, h, sl], ef[:], er[:, h, sl].bitcast(F32), op=ALU.subtract
                    )
            e2b = cp.tile([4, K], BF16, tag="e2b")
            nc.sync.dma_start(e2b[:], e2q[:])
            ones4 = cp.tile([4, 128], BF16, tag="ones4")
            nc.gpsimd.memset(ones4[:], 1.0)
            idn = cp.tile([128, 128], F32, tag="idn")
            masks.make_identity(nc, idn[:])

            # ---------------- acoustic path (thirds) ----------------
            ac_flat = (
                xb[S:D, :]
                .rearrange("a b -> (a b)")
                .rearrange("(p f) -> p f", p=AC_FLAT_P)
            )
            for j in range(3):
                sl = slice(j * AC_THIRD, (j + 1) * AC_THIRD)
                af = acp.tile([128, AC_THIRD], F32, tag="af")
                nc.sync.dma_start(af[:], ac_flat[:, sl])
                th = acp.tile([128, AC_THIRD], F32, tag="th")
                nc.scalar.activation(th[:], af[:], AF.Tanh)
                scl = acp.tile([128, AC_THIRD], F32, tag="scl")
                nc.vector.tensor_scalar(
                    scl[:], th[:], 10.0, 10.0, op0=ALU.mult, op1=ALU.add
                )
                rnd = acp.tile([128, AC_THIRD], F32, tag="rnd")
                nc.vector.tensor_scalar(
                    rnd[:], scl[:], MAGIC, -MAGIC, op0=ALU.add, op1=ALU.add
                )
                cac = acp.tile([128, AC_THIRD], I32, tag="cac")
                nc.vector.tensor_copy(cac[:], rnd[:])
                nc.gpsimd.dma_start(o_cac[:, sl], cac[:])
                rac = acp.tile([128, AC_THIRD], F32, tag="rac")
                nc.vector.tensor_scalar(
                    rac[:], rnd[:], 0.1, -1.0, op0=ALU.mult, op1=ALU.add
                )
                nc.gpsimd.dma_start(o_rac[:, sl], rac[:])

            # ---------------- main loop over 32 sample tiles ----------------
            for i in range(NTILES):
                t0 = i * 128
                # x tile load + precision splits
                xf = xp.tile([128, 2, 128], F32, tag="xf")
                for h in range(2):
                    nc.sync.dma_start(
                        xf[:, h, :], xb[h * 128 : (h + 1) * 128, t0 : t0 + 128]
                    )
                xr = xp.tile([128, 2, 128], F32R, tag="xr")
                nc.vector.tensor_copy(xr[:], xf[:])
                xl = xp.tile([128, 2, 128], F32R, tag="xl")
                nc.vector.tensor_tensor(
                    xl[:], xf[:], xr[:].bitcast(F32), op=ALU.subtract
                )
                xh = xp.tile([128, 2, 128], BF16, tag="xh")
                nc.vector.tensor_copy(xh[:], xf[:])

                c = ccp.tile([128, K], F32, tag="c")
                for q in range(NQ):
                    acc = ps.tile([128, QW], F32, tag="acc")
                    for cc in range(QW // 512):
                        kk = q * QW + cc * 512
                        sl = slice(kk, kk + 512)
                        osl = slice(cc * 512, (cc + 1) * 512)
                        nc.tensor.matmul(
                            acc[:, osl], xr[:, 0, :], er[:, 0, sl],
                            start=True, stop=False,
                        )
                        nc.tensor.matmul(
                            acc[:, osl], xr[:, 1, :], er[:, 1, sl],
                            start=False, stop=False,
                        )
                        nc.tensor.matmul(
                            acc[:, osl], xl[:, 0, :], er[:, 0, sl],
                            start=False, stop=False,
                        )
                        nc.tensor.matmul(
                            acc[:, osl], xl[:, 1, :], er[:, 1, sl],
                            start=False, stop=False,
                        )
                        nc.tensor.matmul(
                            acc[:, osl], xh[:, 0, :], el[:, 0, sl],
                            start=False, stop=False,
                        )
                        nc.tensor.matmul(
                            acc[:, osl], xh[:, 1, :], el[:, 1, sl],
                            start=False, stop=False,
                        )
                        nc.tensor.matmul(
                            acc[:, osl], ones4[:], e2b[:, sl],
                            start=False, stop=True,
                        )
                    # chained prefix-max scan (PSUM -> SBUF)
                    qsl = slice(q * QW, (q + 1) * QW)
                    init = -3.0e38 if q == 0 else c[:, q * QW - 1 : q * QW]
                    nc.vector.tensor_tensor_scan(
                        c[:, qsl], acc[:], el[:, 0, 0:QW], init,
                        op0=ALU.max, op1=ALU.bypass,
                    )

                # counts: index = #(c_j < gmax) over both halves
                gmax = c[:, K - 1 : K]
                sgn = sgp.tile([128, K // 2], BF16, tag="sgn")
                cnta = smp.tile([128, 1], F32, tag="cnta")
                cntb = smp.tile([128, 1], F32, tag="cntb")
                nc.scalar.activation(
                    sgn[:], c[:, 0 : K // 2], AF.Sign,
                    bias=gmax, scale=-1.0, accum_out=cnta[:],
                )
                nc.scalar.activation(
                    sgn[:], c[:, K // 2 : K], AF.Sign,
                    bias=gmax, scale=-1.0, accum_out=cntb[:],
                )
                idxf = smp.tile([128, 1], F32, tag="idxf")
                nc.vector.tensor_tensor(idxf[:], cnta[:], cntb[:], op=ALU.add)
                idxi = smp.tile([128, 1], I32, tag="idxi")
                nc.vector.tensor_copy(idxi[:], idxf[:])
                nc.gpsimd.dma_start(o_csem[t0 : t0 + 128], idxi[:, 0])

                # decode: gather emb rows, transpose to [feat, t]
                gt = gp.tile([128, S], F32, tag="gt")
                nc.gpsimd.indirect_dma_start(
                    gt[:], None, embt[:], IndirectOffsetOnAxis(ap=idxi[:], axis=0)
                )
                tps = pst.tile([128, S], F32, tag="tps")
                nc.tensor.transpose(tps[:, 0:128], gt[:, 0:128], idn[:])
                nc.tensor.transpose(tps[:, 128:256], gt[:, 128:256], idn[:])
                gT = gp.tile([128, S], F32, tag="gT")
                nc.scalar.copy(gT[:], tps[:])
                nc.gpsimd.dma_start(
                    o_rsem[:, t0 : t0 + 128].rearrange("(h p) t -> p h t", h=2),
                    gT[:].rearrange("p (h c) -> p h c", h=2),
                )
    return nc


_CACHED = {}


def _get_program():
    if "nc" not in _CACHED:
        nc = bacc.Bacc("TRN2", target_bir_lowering=False, debug=False)
        _build(nc)
        nc.compile()
        _CACHED["nc"] = nc
    return _CACHED["nc"]


def _prep_shared(embedding_sum: np.ndarray, cluster_usage: np.ndarray):
    emb = (
        embedding_sum.astype(np.float32)
        / np.clip(cluster_usage.astype(np.float32), EPS, None)[:, None]
    )
    e2t = np.ascontiguousarray((2.0 * emb).T)
    e2_64 = (emb.astype(np.float64) ** 2).sum(axis=1)
    r = (-e2_64).astype(np.float32)
    parts = []
    import ml_dtypes

    for _ in range(4):
        p = r.astype(ml_dtypes.bfloat16)
        parts.append(p)
        r = (r - p.astype(np.float32)).astype(np.float32)
    e2q = np.stack(parts, 0)  # bfloat16 [4, K]
    return emb, e2t, e2q


def _run(x, embedding_sum, cluster_usage, trace=False):
    x = np.asarray(x, dtype=np.float32)
    emb, e2t, e2q = _prep_shared(
        np.asarray(embedding_sum), np.asarray(cluster_usage)
    )
    nc = _get_program()
    in_maps = [
        {"xb": np.ascontiguousarray(x[b]), "e2t": e2t, "e2q": e2q, "embt": emb}
        for b in range(B)
    ]
    try:
        res = run_bass_kernel_spmd(nc, in_maps, list(range(B)), trace=trace)
    except ModuleNotFoundError:
        res = run_bass_kernel_spmd(nc, in_maps, list(range(B)), trace=False)

    codes = np.empty((B, 1 + A, T), dtype=np.int32)
    recon = np.empty((B, D, T), dtype=np.float32)
    for b in range(B):
        rb = res.results[b]
        codes[b, 0, :] = rb["o_csem"]
        codes[b, 1:, :] = rb["o_cac"].reshape(A, T)
        recon[b, :S, :] = rb["o_rsem"]
        recon[b, S:, :] = rb["o_rac"].reshape(A, T)
    return (codes, recon), res.exec_time_ns


def kernel(x, embedding_sum, cluster_usage):
    out, _ = _run(x, embedding_sum, cluster_usage, trace=False)
    return out
